# revision 14
# baseline (speedup 1.0000x reference)
"""Trainium2 Bass kernel for Mobile2Former cross-attention block.

Computation (per batch b):
    xf   = x[b].reshape(C, H*W)                      # [64, 3136] keys=values
    q    = (z[b] @ Wq + bq).reshape(heads, M, C)     # [8, 6, 64]
    attn = softmax(q @ xf * C**-0.5, axis=-1)        # [8, 6, 3136]
    res  = attn @ xf.T                               # [8, 6, 64]
    out  = res.transpose(1,0,2).reshape(M, -1) @ Wo + bo + z[b]

Strategy: data-parallel over B across 8 cores (16 batches/core), batches in
pairs (two batches stacked on the 128 SBUF partitions, C=64 each).  All x
traffic is fp8e4m3: xq [c2, n] feeds QK^T directly (96-column matmuls
against a block-diagonal qT2 stationary), and xv is a HOST-side
pre-transposed [n, c2 (+ones)] copy laid out for DoubleRow fp8 matmuls
(K=256 over two 128-chunks of n per pass), so no on-chip transposes or
PSUM->SBUF copies are needed for the AV product, and the softmax
denominator comes free from a ones-column.  Softmax runs without max
subtraction (logits are O(1); scale*16 folded into Wq/bq and divided back
out inside exp).  exp alternates between the Act engine (exact, scale=1/16)
and the Vector engine (Schraudolph bit-trick straight into fp8e4m3 bits).
The dataflow is software-pipelined: AV(w) is emitted two waves after QK(w),
and each pair's output-projection tail is spread across the next pair's
waves, so the in-order engines never head-block on cross-engine latency.
The output projection streams fcl (12 cols) against stationary Wo chunks,
producing out^T in PSUM; host un-transposes.
"""

import sys
from contextlib import ExitStack

import numpy as np

sys.path.insert(0, "/opt/trn_rl_repo")

import concourse.bass as bass
import concourse.tile as tile
from concourse import bacc as bacc_mod
from concourse import mybir
from concourse.bass_utils import run_bass_kernel_spmd

import ml_dtypes

BF16 = ml_dtypes.bfloat16
FP8 = ml_dtypes.float8_e4m3

N_CORES = 8
B, C, H, W = 128, 64, 56, 56
HW = H * W  # 3136
M, D = 6, 192
NH = 8
INNER = NH * C  # 512
BPC = B // N_CORES  # 16 batches per core
NPAIR = BPC // 2  # 8 pairs per core
NCHUNK = (HW + 127) // 128  # 25 (24 full + one 64-wide)
NCP = 12  # DoubleRow chunk-pairs (chunks 0..23)
XV0 = HW  # xv offset inside the combined x tile
XW = HW + 3300  # combined x tile width (6436)

QS = 16.0  # extra q scale folded into Wq/bq; exp divides it back out
# Schraudolph constants for exp(x/16) in fp8e4m3 bit space:
# byte = round(x * 8/(16*ln2) + B8)
A8 = float(8.0 / (16.0 * np.log(2.0)))
B8 = 55.75

# packed const params: pkA (sync ring) = [bqt f32 16B][pk1 fp8 1216B]
# pkB (act ring) = [zbot f32 768B][pk2 bf16 1792B]
PKA_W = 16 + 1216  # 1232
PKB_W = 768 + 1792  # 2560

F32 = mybir.dt.float32
BF = mybir.dt.bfloat16
E4 = mybir.dt.float8e4
I8 = mybir.dt.int8
U8 = mybir.dt.uint8
DR = mybir.MatmulPerfMode.DoubleRow

_CACHE = {}


def _build_nc() -> bass.Bass:
    nc = bacc_mod.Bacc()

    xall_h = nc.declare_dram_parameter("xall", [NPAIR, 128, XW], E4, isOutput=False)
    pka_h = nc.declare_dram_parameter("pka", [128, PKA_W], U8, isOutput=False)
    pkb_h = nc.declare_dram_parameter("pkb", [128, PKB_W], U8, isOutput=False)
    out_h = nc.declare_dram_parameter("out", [96, 192], F32, isOutput=True)

    with tile.TileContext(nc) as tc, ExitStack() as ctx:
        const = ctx.enter_context(tc.tile_pool(name="const", bufs=1))
        x_pool = ctx.enter_context(tc.tile_pool(name="x", bufs=NPAIR))
        ax_pool = ctx.enter_context(tc.tile_pool(name="ax", bufs=3))
        small = ctx.enter_context(tc.tile_pool(name="small", bufs=3))
        at_psum = ctx.enter_context(tc.tile_pool(name="at_ps", bufs=2, space="PSUM"))
        rs_psum = ctx.enter_context(tc.tile_pool(name="rs_ps", bufs=2, space="PSUM"))
        rt_psum = ctx.enter_context(tc.tile_pool(name="rt_ps", bufs=1, space="PSUM"))
        o2_psum = ctx.enter_context(tc.tile_pool(name="o2_ps", bufs=1, space="PSUM"))

        # ---------------- phase 0: loads ----------------
        # SP ring: qproj deps first, then all x (xq then xv per pair).
        x_tiles = [
            x_pool.tile([128, XW], E4, tag="x", name=f"x{p}")
            for p in range(NPAIR)
        ]
        nc.sync.dma_start(out=x_tiles[0][:, 0:XV0], in_=xall_h.ap()[0][:, 0:XV0])
        pka = const.tile([128, PKA_W], U8)
        nc.sync.dma_start(out=pka, in_=pka_h.ap())
        bqt_sb = pka[:, 0:16].bitcast(F32)  # [128, 4]
        pk1 = pka[:, 16 : 16 + 1216].bitcast(E4)
        zt0 = pk1[:, 0:96]
        zt1 = pk1[0:64, 96:192]
        wq0 = pk1[:, 192:704]
        wq1 = pk1[0:64, 704:1216]

        nc.sync.dma_start(
            out=x_tiles[0][:, XV0:XW], in_=xall_h.ap()[0][:, XV0:XW]
        )
        for p in range(1, NPAIR):
            t = x_tiles[p]
            nc.sync.dma_start(out=t[:, 0:XV0], in_=xall_h.ap()[p][:, 0:XV0])
            nc.sync.dma_start(out=t[:, XV0:XW], in_=xall_h.ap()[p][:, XV0:XW])

        # ACT ring: tail-of-pair constants only.
        pkb = const.tile([128, PKB_W], U8)
        nc.scalar.dma_start(out=pkb, in_=pkb_h.ap())
        zbot_sb = pkb[0:96, 0:768].bitcast(F32)  # [96, 192]
        pk2 = pkb[:, 768 : 768 + 1792].bitcast(BF)  # [128, 896]
        ident = pk2[:, 0:128]
        wo_sb = pk2[:, 128:896]

        # Persistent qT2 zero blocks (off-diagonal zeros written once).
        qT2_bufs = []
        for i in range(2):
            t = const.tile([128, 96], E4, name=f"qT2_buf{i}")
            nc.gpsimd.memset(t, 0.0)
            qT2_bufs.append(t)

        # q^T for all 16 local batches: qT_all[i, 6b+m] = ((z @ Wq + bq)*s)^T
        # chunk ii holds INNER rows [128*ii, 128*ii+128)
        qT_all = const.tile([128, 4 * 96], E4)
        for ii in range(4):
            qp = at_psum.tile([128, 1024], F32, tag="at", name=f"qp{ii}")
            nc.tensor.matmul(
                qp[:, 0:96], lhsT=wq0[:, 128 * ii : 128 * ii + 128], rhs=zt0,
                start=True, stop=False,
            )
            nc.tensor.matmul(
                qp[:, 0:96], lhsT=wq1[:, 128 * ii : 128 * ii + 128], rhs=zt1,
                start=False, stop=True,
            )
            nc.vector.tensor_scalar_add(
                out=qT_all[:, 96 * ii : 96 * ii + 96], in0=qp[:, 0:96],
                scalar1=bqt_sb[:, ii : ii + 1],
            )

        # ---------------- per-pair main loop ----------------
        # column order inside a pair: hm2 = 48*b + u, u = 6*h + m.
        # Reference's q reshape is a FLAT view of [M, H*C], so the query row
        # for (h, m) is q_flat[(6h+m)//8, 64*((6h+m)%8) : +64].  With
        # u = 8*t + 2*ii + g: source chunk ii, partition half g, z-row t.
        qT_all_g = qT_all.rearrange("p (hh x) -> p hh x", hh=4)  # [128, 4, 96]

        out_allT = const.tile([96, NPAIR * 24], F32)

        def emit_qt2(p):
            """block-diagonal qT2 [c2, hm2] for pair p (gpsimd copies)."""
            qT2 = qT2_bufs[p % 2]
            # col = 48*b + 8*t + 2*ii + g  ->  view [q, b, ii, t, g]
            qT2_v = qT2.rearrange("q (b t ii g) -> q b ii t g", b=2, t=6, ii=4)
            for b in range(2):
                for g in range(2):
                    dst = qT2_v[64 * b : 64 * b + 64, b, :, :, g]
                    src = qT_all_g[
                        64 * g : 64 * g + 64, :, 12 * p + 6 * b : 12 * p + 6 * b + 6
                    ]
                    nc.gpsimd.tensor_copy(out=dst, in_=src)

        emit_qt2(0)

        # per-pair state for the cross-pair software pipeline
        def make_pair_state(p):
            xt = x_tiles[p]
            return {
                "p": p,
                "xt": xt,
                "xq": xt[:, 0:XV0],
                "xv_dr": xt[:, XV0 : XV0 + 264 * NCP].rearrange(
                    "n (cp t c) -> n cp t c", cp=NCP, t=2
                ),
                "qT2": qT2_bufs[p % 2],
                "rsum": rs_psum.tile([96, 129], F32, tag="rs", name=f"rsum{p}"),
                "ax": {},
            }

        def emit_av(st, w):
            """AV for the 4 chunk-pairs of wave w (DoubleRow)."""
            axw = st["ax"].pop(w).rearrange("n (k t x) -> n k t x", k=4, t=2)
            for k in range(4):
                cp = 4 * w + k
                nc.tensor.matmul(
                    st["rsum"], lhsT=axw[:, k, :, :],
                    rhs=st["xv_dr"][:, cp, :, 0:129],
                    perf_mode=DR, start=(cp == 0), stop=False,
                    skip_group_check=True,
                )

        def emit_av24(st):
            nc.tensor.matmul(
                st["rsum"], lhsT=st["ax24"],
                rhs=st["xt"][0:64, XV0 + 3168 : XV0 + 3297],
                start=False, stop=True, skip_group_check=True,
            )

        def emit_norm(st):
            p = st["p"]
            inv = small.tile([96, 1], F32, tag="inv", name=f"inv{p}")
            nc.vector.reciprocal(out=inv, in_=st["rsum"][:, 128:129])
            r2n = small.tile([96, 128], BF, tag="r2n", name=f"r2n{p}")
            nc.vector.tensor_scalar_mul(out=r2n, in0=st["rsum"][:, 0:128], scalar1=inv)
            st["r2n"] = r2n

        def emit_tail_a(st):
            """rt transpose (PE) + SBUF copy (Act) + fcl shuffle (gpsimd)."""
            p = st["p"]
            rt = rt_psum.tile([128, 96], BF, tag="rt", name=f"rt{p}")
            nc.tensor.transpose(rt, st["r2n"], ident[0:96, 0:96])
            rtc = small.tile([128, 96], BF, tag="rtc", name=f"rtc{p}")
            nc.scalar.copy(out=rtc, in_=rt)
            # fc rhs: fcl[64*hl + c, 12*kk + 6*b + m]
            #       = rt[64*b + c, 48*b + 12*kk + 6*hl + m]   (h = 2*kk + hl)
            fcl = small.tile([128, 48], BF, tag="fcl", name=f"fcl{p}")
            fcl_g = fcl.rearrange("q (kk x) -> q kk x", kk=4)
            rt_v = rtc.rearrange("q (b kk hl m) -> q b kk hl m", b=2, kk=4, hl=2)
            for hl in range(2):
                for b in range(2):
                    dst = fcl_g[64 * hl : 64 * hl + 64, :, 6 * b : 6 * b + 6]
                    src = rt_v[64 * b : 64 * b + 64, b, :, hl, :]
                    nc.gpsimd.tensor_copy(out=dst, in_=src)
            st["fcl"] = fcl

        def emit_tail_b(st):
            """o2T matmuls + out add."""
            p = st["p"]
            fcl = st["fcl"]
            o2T = o2_psum.tile([96, 24], F32, tag="o2", name=f"o2T{p}")
            for h in range(2):
                for kk in range(4):
                    nc.tensor.matmul(
                        out=o2T[:, 12 * h : 12 * h + 12],
                        lhsT=wo_sb[:, 96 * (2 * kk + h) : 96 * (2 * kk + h) + 96],
                        rhs=fcl[:, 12 * kk : 12 * kk + 12],
                        start=(kk == 0), stop=(kk == 3),
                    )
            nc.vector.tensor_add(
                out=out_allT[:, 24 * p : 24 * p + 24], in0=o2T,
                in1=zbot_sb[:, 24 * p : 24 * p + 24],
            )

        exp_flip = [0]

        def emit_exp(at_ap, ax_ap):
            if exp_flip[0] % 2 == 0:
                nc.scalar.activation(
                    out=ax_ap, in_=at_ap, func=mybir.ActivationFunctionType.Exp,
                    scale=1.0 / QS,
                )
            else:
                nc.vector.tensor_scalar(
                    out=ax_ap.bitcast(I8), in0=at_ap, scalar1=A8, scalar2=B8,
                    op0=mybir.AluOpType.mult, op1=mybir.AluOpType.add,
                )
            exp_flip[0] += 1

        # at tile: [128, 1024] f32 = 2 PSUM banks; 4 chunks of 96 + pad per
        # bank.  chunk slot j (0..7) at col 512*(j//4) + 96*(j%4); wave-2's
        # chunk 24 uses the bank-0 pad (cols 384:480).
        def at_col(j):
            return 512 * (j // 4) + 96 * (j % 4)

        prev = None
        for p in range(NPAIR):
            st = make_pair_state(p)
            xq, qT2 = st["xq"], st["qT2"]
            # 3 waves x 8 chunks; pair p's AV(w) runs one wave later; wave
            # 2's AVs + chunk 24 + normalize/projection run in pair p+1.
            for w in range(3):
                at = at_psum.tile([128, 1024], F32, tag="at", name=f"at{p}_{w}")
                for j in range(8):
                    ch = 8 * w + j
                    nc.tensor.matmul(
                        at[:, at_col(j) : at_col(j) + 96],
                        lhsT=xq[:, 128 * ch : 128 * ch + 128], rhs=qT2,
                        start=True, stop=True,
                    )
                if w == 2:
                    nc.tensor.matmul(
                        at[0:64, 384:480], lhsT=xq[:, 3072:3136], rhs=qT2,
                        start=True, stop=True,
                    )
                if prev is not None:
                    if w == 0:
                        emit_av(prev, 2)
                        emit_av24(prev)
                    elif w == 1:
                        emit_norm(prev)
                        emit_tail_a(prev)
                        if p + 1 < NPAIR:
                            emit_qt2(p + 1)
                    elif w == 2:
                        emit_tail_b(prev)
                elif w == 1 and p + 1 < NPAIR:
                    emit_qt2(p + 1)
                ax = ax_pool.tile([128, 768], E4, tag="ax", name=f"ax{p}_{w}")
                emit_exp(
                    at.rearrange("n (b x) -> n b x", b=2)[:, :, 0:384],
                    ax.rearrange("n (b x) -> n b x", b=2),
                )
                if w == 2:
                    ax24 = small.tile([64, 96], E4, tag="ax24", name=f"ax24_{p}")
                    emit_exp(at[0:64, 384:480], ax24)
                    st["ax24"] = ax24
                st["ax"][w] = ax
                if w >= 1:
                    emit_av(st, w - 1)
            prev = st

        # drain the last pair
        emit_av(prev, 2)
        emit_av24(prev)
        emit_norm(prev)
        emit_tail_a(prev)
        emit_tail_b(prev)
        nc.sync.dma_start(out=out_h.ap(), in_=out_allT)

    return nc


def get_nc() -> bass.Bass:
    if "nc" not in _CACHE:
        nc = _build_nc()
        # The PJRT exec path serializes nc.m as-is; run Bacc's legalization
        # (wait splitting, register allocation, ...) explicitly.
        nc.finalize()
        _CACHE["nc"] = nc
    return _CACHE["nc"]


def make_in_maps(x, z, Wq, bq, Wo, bo):
    """Host-side prep + sharding into per-core input maps."""
    x = np.asarray(x, dtype=np.float32)
    z = np.asarray(z, dtype=np.float32)
    Wq = np.asarray(Wq, dtype=np.float32)
    bq = np.asarray(bq, dtype=np.float32)
    Wo = np.asarray(Wo, dtype=np.float32)
    bo = np.asarray(bo, dtype=np.float32)

    scale = np.float32(C ** -0.5 * QS)
    x8 = x.reshape(B, C, HW).astype(FP8)
    wq_s = (Wq * scale).astype(FP8)
    bqt = np.ascontiguousarray((bq * scale).reshape(4, 128).T.astype(np.float32))
    wo_bf = Wo.astype(BF16)

    # pk2 = [ident 128 | wo_sb 768]; wo_sb[p, 96*(2kk+h)+dd] = Wo[128kk+p, 96h+dd]
    pk2 = np.zeros((128, 896), dtype=BF16)
    pk2[:, 0:128] = np.eye(128, dtype=BF16)
    pk2[:, 128:896] = (
        wo_bf.reshape(4, 128, 2, 96).transpose(1, 0, 2, 3).reshape(128, 768)
    )

    # xv: [pair][n-part, 12*(2*132) + 132] pre-transposed fp8 with ones col
    # DR region: col = 264*cp + 132*t + cc ; n = 128*(2cp+t) + npart
    npairs = NPAIR * N_CORES
    xt = np.zeros((npairs, 3328, 130), dtype=np.float32)
    xpairs = x.reshape(npairs, 2, C, HW)
    xt[:, :HW, :128] = np.transpose(xpairs, (0, 3, 1, 2)).reshape(npairs, HW, 128)
    xt[:, :HW, 128] = 1.0
    xv_dr = (
        xt[:, : 128 * 24]
        .reshape(npairs, NCP, 2, 128, 130)
        .transpose(0, 3, 1, 2, 4)
    )  # [pair, npart, cp, t, 130]
    xall = np.zeros((npairs, 128, XW), dtype=FP8)
    xall[:, :, 0:HW] = x8.reshape(npairs, 128, HW)
    dr_block = np.zeros((npairs, 128, NCP, 2, 132), dtype=FP8)
    dr_block[:, :, :, :, 0:130] = xv_dr.astype(FP8)
    xall[:, :, XV0 : XV0 + 264 * NCP] = dr_block.reshape(npairs, 128, 264 * NCP)
    # chunk 24: xv[:, npart, 3168+cc] = xt[:, 3072+npart, cc] (zeros beyond HW)
    xall[:, :, XV0 + 3168 : XV0 + 3298] = xt[:, 3072:3200, :].astype(FP8)

    # zbot[dd, 24p+12h+6bl+m] = z[2p+bl, m, 96h+dd] + bo[96h+dd]  (per core)
    zbo = z + bo[None, None, :]  # [B, M, D]

    in_maps = []
    for i in range(N_CORES):
        s = slice(i * BPC, (i + 1) * BPC)
        ps = slice(i * NPAIR, (i + 1) * NPAIR)
        # zt[d, 6*b_local + m] = z[core_base + b_local, m, d] (fp8)
        zt = z[s].reshape(BPC * M, D).T.astype(FP8)
        pk1 = np.zeros((128, 1216), dtype=FP8)
        pk1[:, 0:96] = zt[0:128]
        pk1[0:64, 96:192] = zt[128:192]
        pk1[:, 192:704] = wq_s[0:128]
        pk1[0:64, 704:1216] = wq_s[128:192]
        # [pair, bl, m, h, dd] -> [dd, pair, h, bl, m]
        zbot = (
            zbo[s]
            .reshape(NPAIR, 2, M, 2, 96)
            .transpose(4, 0, 3, 1, 2)
            .reshape(96, 192)
        )
        pka = np.zeros((128, PKA_W), dtype=np.uint8)
        pka[:, 0:16] = bqt.view(np.uint8)
        pka[:, 16:] = pk1.view(np.uint8)
        pkb = np.zeros((128, PKB_W), dtype=np.uint8)
        pkb[0:96, 0:768] = zbot.astype(np.float32).view(np.uint8)
        pkb[:, 768:] = pk2.view(np.uint8)
        in_maps.append(
            {
                "xall": np.ascontiguousarray(xall[ps]),
                "pka": pka,
                "pkb": pkb,
            }
        )
    return in_maps


def gather_out(res) -> np.ndarray:
    # out[dd, 24p+12h+6bl+m] -> [p, bl, m, 2, 96]
    outs = []
    for i in range(N_CORES):
        o = np.asarray(res.results[i]["out"], dtype=np.float32)
        o = o.reshape(96, NPAIR, 2, 2, M).transpose(1, 3, 4, 2, 0).reshape(
            BPC, M, D
        )
        outs.append(o)
    return np.concatenate(outs, axis=0).astype(np.float32)


def kernel(**inputs) -> np.ndarray:
    nc = get_nc()
    in_maps = make_in_maps(
        inputs["x"], inputs["z"], inputs["Wq"], inputs["bq"],
        inputs["Wo"], inputs["bo"],
    )
    res = run_bass_kernel_spmd(nc, in_maps, list(range(N_CORES)))
    return gather_out(res)


# revision 15
# speedup vs baseline: 1.0805x; 1.0805x over previous
"""Trainium2 Bass kernel for Mobile2Former cross-attention block.

Computation (per batch b):
    xf   = x[b].reshape(C, H*W)                      # [64, 3136] keys=values
    q    = (z[b] @ Wq + bq).reshape(heads, M, C)     # [8, 6, 64]
    attn = softmax(q @ xf * C**-0.5, axis=-1)        # [8, 6, 3136]
    res  = attn @ xf.T                               # [8, 6, 64]
    out  = res.transpose(1,0,2).reshape(M, -1) @ Wo + bo + z[b]

Strategy: data-parallel over B across 8 cores (16 batches/core), batches in
pairs (two batches stacked on the 128 SBUF partitions, C=64 each).  All x
traffic is fp8e4m3: xq [c2, n] feeds QK^T directly (96-column matmuls
against a block-diagonal qT2 stationary), and xv is a HOST-side
pre-transposed [n, c2 (+ones)] copy laid out for DoubleRow fp8 matmuls
(K=256 over two 128-chunks of n per pass), so no on-chip transposes or
PSUM->SBUF copies are needed for the AV product, and the softmax
denominator comes free from a ones-column.  Softmax runs without max
subtraction (logits are O(1); scale*16 folded into Wq/bq and divided back
out inside exp).  exp alternates between the Act engine (exact, scale=1/16)
and the Vector engine (Schraudolph bit-trick straight into fp8e4m3 bits).
The dataflow is software-pipelined: AV(w) is emitted two waves after QK(w),
and each pair's output-projection tail is spread across the next pair's
waves, so the in-order engines never head-block on cross-engine latency.
The output projection streams fcl (12 cols) against stationary Wo chunks,
producing out^T in PSUM; host un-transposes.
"""

import sys
from contextlib import ExitStack

import numpy as np

sys.path.insert(0, "/opt/trn_rl_repo")

import concourse.bass as bass
import concourse.tile as tile
from concourse import bacc as bacc_mod
from concourse import mybir
from concourse.bass_utils import run_bass_kernel_spmd

import ml_dtypes

BF16 = ml_dtypes.bfloat16
FP8 = ml_dtypes.float8_e4m3

N_CORES = 8
B, C, H, W = 128, 64, 56, 56
HW = H * W  # 3136
M, D = 6, 192
NH = 8
INNER = NH * C  # 512
BPC = B // N_CORES  # 16 batches per core
NPAIR = BPC // 2  # 8 pairs per core
NCHUNK = (HW + 127) // 128  # 25 (24 full + one 64-wide)
NCP = 12  # DoubleRow chunk-pairs (chunks 0..23)
XV0 = HW  # xv offset inside the combined x tile
XW = HW + 3300  # combined x tile width (6436)

QS = 16.0  # extra q scale folded into Wq/bq; exp divides it back out
# Schraudolph constants for exp(x/16) in fp8e4m3 bit space:
# byte = round(x * 8/(16*ln2) + B8)
A8 = float(8.0 / (16.0 * np.log(2.0)))
B8 = 55.75

# packed const params: pkA (sync ring) = [bqt f32 16B][pk1 fp8 1216B]
# pkB (act ring) = [zbot f32 768B][pk2 bf16 1792B]
PKA_W = 16 + 1216  # 1232
PKB_W = 768 + 1792  # 2560

F32 = mybir.dt.float32
BF = mybir.dt.bfloat16
E4 = mybir.dt.float8e4
I8 = mybir.dt.int8
U8 = mybir.dt.uint8
DR = mybir.MatmulPerfMode.DoubleRow

_CACHE = {}


def _build_nc() -> bass.Bass:
    nc = bacc_mod.Bacc()

    xall_h = nc.declare_dram_parameter("xall", [NPAIR, 128, XW], E4, isOutput=False)
    pka_h = nc.declare_dram_parameter("pka", [128, PKA_W], U8, isOutput=False)
    pkb_h = nc.declare_dram_parameter("pkb", [128, PKB_W], U8, isOutput=False)
    out_h = nc.declare_dram_parameter("out", [96, 192], F32, isOutput=True)

    with tile.TileContext(nc) as tc, ExitStack() as ctx:
        const = ctx.enter_context(tc.tile_pool(name="const", bufs=1))
        x_pool = ctx.enter_context(tc.tile_pool(name="x", bufs=NPAIR))
        ax_pool = ctx.enter_context(tc.tile_pool(name="ax", bufs=4))
        small = ctx.enter_context(tc.tile_pool(name="small", bufs=3))
        at_psum = ctx.enter_context(tc.tile_pool(name="at_ps", bufs=4, space="PSUM"))
        rs_psum = ctx.enter_context(tc.tile_pool(name="rs_ps", bufs=2, space="PSUM"))
        rt_psum = ctx.enter_context(tc.tile_pool(name="rt_ps", bufs=1, space="PSUM"))
        o2_psum = ctx.enter_context(tc.tile_pool(name="o2_ps", bufs=1, space="PSUM"))

        # ---------------- phase 0: loads ----------------
        # SP ring: qproj deps first, then all x (xq then xv per pair).
        x_tiles = [
            x_pool.tile([128, XW], E4, tag="x", name=f"x{p}")
            for p in range(NPAIR)
        ]
        nc.sync.dma_start(out=x_tiles[0][:, 0:XV0], in_=xall_h.ap()[0][:, 0:XV0])
        pka = const.tile([128, PKA_W], U8)
        nc.sync.dma_start(out=pka, in_=pka_h.ap())
        bqt_sb = pka[:, 0:16].bitcast(F32)  # [128, 4]
        pk1 = pka[:, 16 : 16 + 1216].bitcast(E4)
        zt0 = pk1[:, 0:96]
        zt1 = pk1[0:64, 96:192]
        wq0 = pk1[:, 192:704]
        wq1 = pk1[0:64, 704:1216]

        nc.sync.dma_start(
            out=x_tiles[0][:, XV0:XW], in_=xall_h.ap()[0][:, XV0:XW]
        )
        for p in range(1, NPAIR):
            t = x_tiles[p]
            nc.sync.dma_start(out=t[:, 0:XV0], in_=xall_h.ap()[p][:, 0:XV0])
            nc.sync.dma_start(out=t[:, XV0:XW], in_=xall_h.ap()[p][:, XV0:XW])

        # ACT ring: tail-of-pair constants only.
        pkb = const.tile([128, PKB_W], U8)
        nc.scalar.dma_start(out=pkb, in_=pkb_h.ap())
        zbot_sb = pkb[0:96, 0:768].bitcast(F32)  # [96, 192]
        pk2 = pkb[:, 768 : 768 + 1792].bitcast(BF)  # [128, 896]
        ident = pk2[:, 0:128]
        wo_sb = pk2[:, 128:896]

        # Persistent qT2 zero blocks (off-diagonal zeros written once).
        qT2_bufs = []
        for i in range(2):
            t = const.tile([128, 96], E4, name=f"qT2_buf{i}")
            nc.gpsimd.memset(t, 0.0)
            qT2_bufs.append(t)

        # q^T for all 16 local batches: qT_all[i, 6b+m] = ((z @ Wq + bq)*s)^T
        # chunk ii holds INNER rows [128*ii, 128*ii+128)
        qT_all = const.tile([128, 4 * 96], E4)
        for ii in range(4):
            qp = at_psum.tile([128, 384], F32, tag="at", name=f"qp{ii}")
            nc.tensor.matmul(
                qp[:, 0:96], lhsT=wq0[:, 128 * ii : 128 * ii + 128], rhs=zt0,
                start=True, stop=False,
            )
            nc.tensor.matmul(
                qp[:, 0:96], lhsT=wq1[:, 128 * ii : 128 * ii + 128], rhs=zt1,
                start=False, stop=True,
            )
            nc.vector.tensor_scalar_add(
                out=qT_all[:, 96 * ii : 96 * ii + 96], in0=qp[:, 0:96],
                scalar1=bqt_sb[:, ii : ii + 1],
            )

        # ---------------- per-pair main loop ----------------
        # column order inside a pair: hm2 = 48*b + u, u = 6*h + m.
        # Reference's q reshape is a FLAT view of [M, H*C], so the query row
        # for (h, m) is q_flat[(6h+m)//8, 64*((6h+m)%8) : +64].  With
        # u = 8*t + 2*ii + g: source chunk ii, partition half g, z-row t.
        qT_all_g = qT_all.rearrange("p (hh x) -> p hh x", hh=4)  # [128, 4, 96]

        out_allT = const.tile([96, NPAIR * 24], F32)

        def emit_qt2(p):
            """block-diagonal qT2 [c2, hm2] for pair p (gpsimd copies)."""
            qT2 = qT2_bufs[p % 2]
            # col = 48*b + 8*t + 2*ii + g  ->  view [q, b, ii, t, g]
            qT2_v = qT2.rearrange("q (b t ii g) -> q b ii t g", b=2, t=6, ii=4)
            for b in range(2):
                for g in range(2):
                    dst = qT2_v[64 * b : 64 * b + 64, b, :, :, g]
                    src = qT_all_g[
                        64 * g : 64 * g + 64, :, 12 * p + 6 * b : 12 * p + 6 * b + 6
                    ]
                    nc.gpsimd.tensor_copy(out=dst, in_=src)

        emit_qt2(0)

        # per-pair state for the cross-pair software pipeline
        def make_pair_state(p):
            xt = x_tiles[p]
            return {
                "p": p,
                "xt": xt,
                "xq": xt[:, 0:XV0],
                "xv_dr": xt[:, XV0 : XV0 + 264 * NCP].rearrange(
                    "n (cp t c) -> n cp t c", cp=NCP, t=2
                ),
                "qT2": qT2_bufs[p % 2],
                "rsum": rs_psum.tile([96, 129], F32, tag="rs", name=f"rsum{p}"),
                "ax": {},
            }

        def emit_av(st, w):
            """AV for the 2 chunk-pairs of wave w (DoubleRow)."""
            axw = st["ax"].pop(w).rearrange("n (k t x) -> n k t x", k=2, t=2)
            for k in range(2):
                cp = 2 * w + k
                nc.tensor.matmul(
                    st["rsum"], lhsT=axw[:, k, :, :],
                    rhs=st["xv_dr"][:, cp, :, 0:129],
                    perf_mode=DR, start=(cp == 0), stop=False,
                    skip_group_check=True,
                )

        def emit_av24(st):
            nc.tensor.matmul(
                st["rsum"], lhsT=st["ax24"],
                rhs=st["xt"][0:64, XV0 + 3168 : XV0 + 3297],
                start=False, stop=True, skip_group_check=True,
            )

        def emit_norm(st):
            p = st["p"]
            inv = small.tile([96, 1], F32, tag="inv", name=f"inv{p}")
            nc.vector.reciprocal(out=inv, in_=st["rsum"][:, 128:129])
            r2n = small.tile([96, 128], BF, tag="r2n", name=f"r2n{p}")
            nc.vector.tensor_scalar_mul(out=r2n, in0=st["rsum"][:, 0:128], scalar1=inv)
            st["r2n"] = r2n

        def emit_tail_a(st):
            """rt transpose (PE) + SBUF copy (Act) + fcl shuffle (gpsimd)."""
            p = st["p"]
            rt = rt_psum.tile([128, 96], BF, tag="rt", name=f"rt{p}")
            nc.tensor.transpose(rt, st["r2n"], ident[0:96, 0:96])
            rtc = small.tile([128, 96], BF, tag="rtc", name=f"rtc{p}")
            nc.scalar.copy(out=rtc, in_=rt)
            # fc rhs: fcl[64*hl + c, 12*kk + 6*b + m]
            #       = rt[64*b + c, 48*b + 12*kk + 6*hl + m]   (h = 2*kk + hl)
            fcl = small.tile([128, 48], BF, tag="fcl", name=f"fcl{p}")
            fcl_g = fcl.rearrange("q (kk x) -> q kk x", kk=4)
            rt_v = rtc.rearrange("q (b kk hl m) -> q b kk hl m", b=2, kk=4, hl=2)
            for hl in range(2):
                for b in range(2):
                    dst = fcl_g[64 * hl : 64 * hl + 64, :, 6 * b : 6 * b + 6]
                    src = rt_v[64 * b : 64 * b + 64, b, :, hl, :]
                    nc.gpsimd.tensor_copy(out=dst, in_=src)
            st["fcl"] = fcl

        def emit_tail_b(st):
            """o2T matmuls + out add."""
            p = st["p"]
            fcl = st["fcl"]
            o2T = o2_psum.tile([96, 24], F32, tag="o2", name=f"o2T{p}")
            for h in range(2):
                for kk in range(4):
                    nc.tensor.matmul(
                        out=o2T[:, 12 * h : 12 * h + 12],
                        lhsT=wo_sb[:, 96 * (2 * kk + h) : 96 * (2 * kk + h) + 96],
                        rhs=fcl[:, 12 * kk : 12 * kk + 12],
                        start=(kk == 0), stop=(kk == 3),
                    )
            nc.vector.tensor_add(
                out=out_allT[:, 24 * p : 24 * p + 24], in0=o2T,
                in1=zbot_sb[:, 24 * p : 24 * p + 24],
            )

        exp_flip = [0]

        def emit_exp(at_ap, ax_ap):
            if exp_flip[0] % 2 == 0:
                nc.scalar.activation(
                    out=ax_ap, in_=at_ap, func=mybir.ActivationFunctionType.Exp,
                    scale=1.0 / QS,
                )
            else:
                nc.vector.tensor_scalar(
                    out=ax_ap.bitcast(I8), in0=at_ap, scalar1=A8, scalar2=B8,
                    op0=mybir.AluOpType.mult, op1=mybir.AluOpType.add,
                )
            exp_flip[0] += 1

        prev = None
        for p in range(NPAIR):
            st = make_pair_state(p)
            xq, qT2 = st["xq"], st["qT2"]
            # 6 waves x 2 chunk-pairs; pair p's AV(w) runs one wave later;
            # wave 5's AVs + chunk 24 + normalize/projection run in pair p+1.
            for w in range(6):
                at = at_psum.tile([128, 384], F32, tag="at", name=f"at{p}_{w}")
                for j in range(4):
                    ch = 4 * w + j
                    nc.tensor.matmul(
                        at[:, 96 * j : 96 * j + 96],
                        lhsT=xq[:, 128 * ch : 128 * ch + 128], rhs=qT2,
                        start=True, stop=True,
                    )
                if prev is not None:
                    if w == 0:
                        emit_av(prev, 5)
                        emit_av24(prev)
                    elif w == 1:
                        emit_norm(prev)
                    elif w == 2:
                        emit_tail_a(prev)
                        if p + 1 < NPAIR:
                            emit_qt2(p + 1)
                    elif w == 4:
                        emit_tail_b(prev)
                elif w == 1 and p + 1 < NPAIR:
                    emit_qt2(p + 1)
                ax = ax_pool.tile([128, 384], E4, tag="ax", name=f"ax{p}_{w}")
                emit_exp(at, ax)
                if w == 5:
                    at24 = at_psum.tile(
                        [128, 384], F32, tag="at", name=f"at24_{p}"
                    )
                    nc.tensor.matmul(
                        at24[0:64, 0:96], lhsT=xq[:, 3072:3136], rhs=qT2,
                        start=True, stop=True,
                    )
                    ax24 = small.tile([64, 96], E4, tag="ax24", name=f"ax24_{p}")
                    emit_exp(at24[0:64, 0:96], ax24)
                    st["ax24"] = ax24
                st["ax"][w] = ax
                if w >= 1:
                    emit_av(st, w - 1)
            prev = st

        # drain the last pair
        emit_av(prev, 5)
        emit_av24(prev)
        emit_norm(prev)
        emit_tail_a(prev)
        emit_tail_b(prev)
        nc.sync.dma_start(out=out_h.ap(), in_=out_allT)

    return nc


def get_nc() -> bass.Bass:
    if "nc" not in _CACHE:
        nc = _build_nc()
        # The PJRT exec path serializes nc.m as-is; run Bacc's legalization
        # (wait splitting, register allocation, ...) explicitly.
        nc.finalize()
        _CACHE["nc"] = nc
    return _CACHE["nc"]


def make_in_maps(x, z, Wq, bq, Wo, bo):
    """Host-side prep + sharding into per-core input maps."""
    x = np.asarray(x, dtype=np.float32)
    z = np.asarray(z, dtype=np.float32)
    Wq = np.asarray(Wq, dtype=np.float32)
    bq = np.asarray(bq, dtype=np.float32)
    Wo = np.asarray(Wo, dtype=np.float32)
    bo = np.asarray(bo, dtype=np.float32)

    scale = np.float32(C ** -0.5 * QS)
    x8 = x.reshape(B, C, HW).astype(FP8)
    wq_s = (Wq * scale).astype(FP8)
    bqt = np.ascontiguousarray((bq * scale).reshape(4, 128).T.astype(np.float32))
    wo_bf = Wo.astype(BF16)

    # pk2 = [ident 128 | wo_sb 768]; wo_sb[p, 96*(2kk+h)+dd] = Wo[128kk+p, 96h+dd]
    pk2 = np.zeros((128, 896), dtype=BF16)
    pk2[:, 0:128] = np.eye(128, dtype=BF16)
    pk2[:, 128:896] = (
        wo_bf.reshape(4, 128, 2, 96).transpose(1, 0, 2, 3).reshape(128, 768)
    )

    # xv: [pair][n-part, 12*(2*132) + 132] pre-transposed fp8 with ones col
    # DR region: col = 264*cp + 132*t + cc ; n = 128*(2cp+t) + npart
    npairs = NPAIR * N_CORES
    xt = np.zeros((npairs, 3328, 130), dtype=np.float32)
    xpairs = x.reshape(npairs, 2, C, HW)
    xt[:, :HW, :128] = np.transpose(xpairs, (0, 3, 1, 2)).reshape(npairs, HW, 128)
    xt[:, :HW, 128] = 1.0
    xv_dr = (
        xt[:, : 128 * 24]
        .reshape(npairs, NCP, 2, 128, 130)
        .transpose(0, 3, 1, 2, 4)
    )  # [pair, npart, cp, t, 130]
    xall = np.zeros((npairs, 128, XW), dtype=FP8)
    xall[:, :, 0:HW] = x8.reshape(npairs, 128, HW)
    dr_block = np.zeros((npairs, 128, NCP, 2, 132), dtype=FP8)
    dr_block[:, :, :, :, 0:130] = xv_dr.astype(FP8)
    xall[:, :, XV0 : XV0 + 264 * NCP] = dr_block.reshape(npairs, 128, 264 * NCP)
    # chunk 24: xv[:, npart, 3168+cc] = xt[:, 3072+npart, cc] (zeros beyond HW)
    xall[:, :, XV0 + 3168 : XV0 + 3298] = xt[:, 3072:3200, :].astype(FP8)

    # zbot[dd, 24p+12h+6bl+m] = z[2p+bl, m, 96h+dd] + bo[96h+dd]  (per core)
    zbo = z + bo[None, None, :]  # [B, M, D]

    in_maps = []
    for i in range(N_CORES):
        s = slice(i * BPC, (i + 1) * BPC)
        ps = slice(i * NPAIR, (i + 1) * NPAIR)
        # zt[d, 6*b_local + m] = z[core_base + b_local, m, d] (fp8)
        zt = z[s].reshape(BPC * M, D).T.astype(FP8)
        pk1 = np.zeros((128, 1216), dtype=FP8)
        pk1[:, 0:96] = zt[0:128]
        pk1[0:64, 96:192] = zt[128:192]
        pk1[:, 192:704] = wq_s[0:128]
        pk1[0:64, 704:1216] = wq_s[128:192]
        # [pair, bl, m, h, dd] -> [dd, pair, h, bl, m]
        zbot = (
            zbo[s]
            .reshape(NPAIR, 2, M, 2, 96)
            .transpose(4, 0, 3, 1, 2)
            .reshape(96, 192)
        )
        pka = np.zeros((128, PKA_W), dtype=np.uint8)
        pka[:, 0:16] = bqt.view(np.uint8)
        pka[:, 16:] = pk1.view(np.uint8)
        pkb = np.zeros((128, PKB_W), dtype=np.uint8)
        pkb[0:96, 0:768] = zbot.astype(np.float32).view(np.uint8)
        pkb[:, 768:] = pk2.view(np.uint8)
        in_maps.append(
            {
                "xall": np.ascontiguousarray(xall[ps]),
                "pka": pka,
                "pkb": pkb,
            }
        )
    return in_maps


def gather_out(res) -> np.ndarray:
    # out[dd, 24p+12h+6bl+m] -> [p, bl, m, 2, 96]
    outs = []
    for i in range(N_CORES):
        o = np.asarray(res.results[i]["out"], dtype=np.float32)
        o = o.reshape(96, NPAIR, 2, 2, M).transpose(1, 3, 4, 2, 0).reshape(
            BPC, M, D
        )
        outs.append(o)
    return np.concatenate(outs, axis=0).astype(np.float32)


def kernel(**inputs) -> np.ndarray:
    nc = get_nc()
    in_maps = make_in_maps(
        inputs["x"], inputs["z"], inputs["Wq"], inputs["bq"],
        inputs["Wo"], inputs["bo"],
    )
    res = run_bass_kernel_spmd(nc, in_maps, list(range(N_CORES)))
    return gather_out(res)
